# revision 3
# baseline (speedup 1.0000x reference)
"""2-layer GCN (GCNConv -> BatchNorm(train) -> ReLU -> GCNConv -> ReLU) on 8 TRN2
NeuronCores, SPMD (one NEFF on all cores).

Sharding: nodes padded 100000 -> 102400 = 8*12800, core i owns rows
[i*12800,(i+1)*12800); edges partitioned by dst owner so the segment-sum scatter
is local; small 128x128 weights replicated; the layer-2 feature table is
assembled with an AllGather; BatchNorm stats with a 1KB AllReduce.

Per-core pipeline:
  A) H1s table = (dis*x)@W1 for ALL nodes (replicated compute, node-major f32
     in local DRAM); self-loop term dis^2*(x@W1) for own rows seeds the
     aggregation accumulator.
  B) per-edge rows fetched with gpsimd.dma_gather (int16 idx over 4 base-offset
     blocks of 32768 rows, 4 SWDGE queues); segment-sum scatter = one-hot
     matmul accumulated in PSUM per 128-dst chunk.  dis[src]*dis[dst] is
     separable: tables carry the src factor, output rows the dst factor.
     b1 is dropped (BatchNorm output is invariant to a pre-BN bias).
  C) BN stats via ones-matmul partition reduction + AllReduce; affine+ReLU
     fused into one scalar-engine activation in transposed space.
  D) H2s own rows = dis*(h2@W2); AllGather -> full layer-2 table.
  E) same gather/scatter for layer 2 (+b2, ReLU) -> own output rows.
"""
import numpy as np

import concourse.bass as bass
import concourse.mybir as mybir
import concourse.tile as tile
from concourse import bacc
from concourse.bass_utils import run_bass_kernel_spmd
from concourse.masks import make_identity

N = 100000
F = 128
NCORES = 8
NPAD = 102400
OWN = NPAD // NCORES          # 12800
CHUNKS = OWN // 128           # 100
BLK = 32768
NBLK = 4
BN_EPS = 1e-5
GROUPS = NPAD // 128          # 800
MAX_IDX_PER_CALL = 1024

LAST_EXEC_NS = None
LAST_RESULT = None
_cache = {}


def _prep(x, edge_index):
    src = np.asarray(edge_index[0]).astype(np.int64)
    dst = np.asarray(edge_index[1]).astype(np.int64)

    deg = np.bincount(dst, minlength=N).astype(np.float32) + 1.0
    dis = np.zeros(NPAD, dtype=np.float32)
    dis[:N] = 1.0 / np.sqrt(deg)

    xs = np.zeros((NPAD, F), dtype=np.float32)
    xs[:N] = np.asarray(x, dtype=np.float32) * dis[:N, None]
    xsT = np.ascontiguousarray(xs.T)                       # [128, NPAD]

    owner = dst // OWN
    chunk = (dst % OWN) // 128
    blk = src // BLK
    cell = ((owner * CHUNKS + chunk) * NBLK + blk).astype(np.int64)
    order = np.argsort(cell, kind="stable")
    cell_s = cell[order]
    src_s = src[order]
    dstloc_s = (dst[order] % 128).astype(np.float32)

    counts = np.bincount(cell_s, minlength=NCORES * CHUNKS * NBLK)
    counts = counts.reshape(NCORES, CHUNKS, NBLK)
    C = counts.max(axis=0)
    C = ((C + 127) // 128) * 128
    C = np.maximum(C, 128)
    slots_per_chunk = C.sum(axis=1)
    tot_slots = int(slots_per_chunk.sum())
    ntiles = tot_slots // 128

    cell_off = np.zeros((CHUNKS, NBLK), dtype=np.int64)
    cell_off.reshape(-1)[1:] = np.cumsum(C.reshape(-1))[:-1]

    starts = np.zeros(NCORES * CHUNKS * NBLK + 1, dtype=np.int64)
    starts[1:] = np.cumsum(counts.reshape(-1))

    per_core = []
    for i in range(NCORES):
        srcidx = np.zeros(tot_slots, dtype=np.int16)          # pads gather row 0
        dstloc = np.full(tot_slots, -1.0, dtype=np.float32)   # pads hit no column
        for c in range(CHUNKS):
            for b in range(NBLK):
                k = (i * CHUNKS + c) * NBLK + b
                n = int(counts[i, c, b])
                o = int(cell_off[c, b])
                if n:
                    sl = slice(starts[k], starts[k] + n)
                    srcidx[o:o + n] = (src_s[sl] - b * BLK).astype(np.int16)
                    dstloc[o:o + n] = dstloc_s[sl]
        iw = srcidx.reshape(tot_slots // 16, 16).T            # [16, tot/16]
        srcidx_w = np.ascontiguousarray(np.tile(iw, (8, 1)))  # [128, tot/16]
        dstloc_t = np.ascontiguousarray(dstloc.reshape(ntiles, 128).T)
        disT = np.ascontiguousarray(
            dis[i * OWN:(i + 1) * OWN].reshape(CHUNKS, 128).T)
        xs_ownT = np.ascontiguousarray(xsT[:, i * OWN:(i + 1) * OWN])
        per_core.append({"srcidx": srcidx_w, "dstloc": dstloc_t,
                         "disT": disT, "xs_ownT": xs_ownT})

    consts = {"C": C, "cell_off": cell_off, "tot_slots": tot_slots,
              "ntiles": ntiles, "slots_per_chunk": slots_per_chunk}
    return consts, xsT, per_core


def _build(consts):
    C = consts["C"]
    cell_off = consts["cell_off"]
    tot_slots = consts["tot_slots"]
    ntiles = consts["ntiles"]
    spc = consts["slots_per_chunk"]

    f32 = mybir.dt.float32
    AF = mybir.ActivationFunctionType
    OP = mybir.AluOpType
    nc = bacc.Bacc("TRN2", target_bir_lowering=False, debug=False,
                   num_devices=NCORES, num_swdge_queues=4)

    xsT_d = nc.dram_tensor("xsT", [F, NPAD], f32, kind="ExternalInput").ap()
    xso_d = nc.dram_tensor("xs_ownT", [F, OWN], f32, kind="ExternalInput").ap()
    W1_d = nc.dram_tensor("W1", [F, F], f32, kind="ExternalInput").ap()
    W2_d = nc.dram_tensor("W2", [F, F], f32, kind="ExternalInput").ap()
    gamma_d = nc.dram_tensor("gamma_c", [F, 1], f32, kind="ExternalInput").ap()
    beta_d = nc.dram_tensor("beta_c", [F, 1], f32, kind="ExternalInput").ap()
    b2m_d = nc.dram_tensor("b2_mat", [128, F], f32, kind="ExternalInput").ap()
    disT_d = nc.dram_tensor("disT", [128, CHUNKS], f32, kind="ExternalInput").ap()
    srcidx_d = nc.dram_tensor("srcidx", [128, tot_slots // 16], mybir.dt.int16,
                              kind="ExternalInput").ap()
    dstloc_d = nc.dram_tensor("dstloc", [128, ntiles], f32,
                              kind="ExternalInput").ap()
    out_d = nc.dram_tensor("out", [OWN, F], f32, kind="ExternalOutput").ap()

    h1s_t = nc.dram_tensor("h1s_tab", [NPAD, F], f32)
    ag_in = nc.dram_tensor("ag_in", [OWN, F], f32)
    ag_out = nc.dram_tensor("ag_out", [NPAD, F], f32, addr_space="Shared")
    bn_in = nc.dram_tensor("bn_in", [F, 2], f32)
    bn_out = nc.dram_tensor("bn_out", [F, 2], f32, addr_space="Shared")

    with tile.TileContext(nc) as tc:
        with tc.tile_pool(name="const", bufs=1) as constp, \
             tc.tile_pool(name="big", bufs=1) as bigp, \
             tc.tile_pool(name="xs", bufs=4) as xsp, \
             tc.tile_pool(name="h", bufs=4) as hp, \
             tc.tile_pool(name="psg", bufs=2, space="PSUM") as psg, \
             tc.tile_pool(name="psb", bufs=4, space="PSUM") as psb, \
             tc.tile_pool(name="pss", bufs=1, space="PSUM") as pss, \
             tc.tile_pool(name="gbuf", bufs=3) as gbufp, \
             tc.tile_pool(name="oh", bufs=8) as ohp, \
             tc.tile_pool(name="wk", bufs=4) as wp, \
             tc.tile_pool(name="st", bufs=1) as stp:

            # ---- constants ----
            W1_t = constp.tile([F, F], f32)
            W2_t = constp.tile([F, F], f32)
            ident = constp.tile([128, 128], f32)
            iota_r = constp.tile([128, 128], f32)
            ones_c = constp.tile([128, 1], f32)
            gamma_t = constp.tile([F, 1], f32)
            beta_t = constp.tile([F, 1], f32)
            b2m_t = constp.tile([128, F], f32)
            disT_t = constp.tile([128, CHUNKS], f32)
            nc.sync.dma_start(out=W1_t[:], in_=W1_d[:])
            nc.sync.dma_start(out=W2_t[:], in_=W2_d[:])
            nc.sync.dma_start(out=gamma_t[:], in_=gamma_d[:])
            nc.sync.dma_start(out=beta_t[:], in_=beta_d[:])
            nc.sync.dma_start(out=b2m_t[:], in_=b2m_d[:])
            nc.sync.dma_start(out=disT_t[:], in_=disT_d[:])
            make_identity(nc, ident[:])
            iota_i = constp.tile([128, 128], mybir.dt.int32)
            nc.gpsimd.iota(iota_i[:], pattern=[[1, 128]], base=0,
                           channel_multiplier=0)
            nc.vector.tensor_copy(out=iota_r[:], in_=iota_i[:])
            nc.vector.memset(ones_c[:], 1.0)

            srcidx_sb = bigp.tile([128, tot_slots // 16], mybir.dt.int16)
            dstloc_sb = bigp.tile([128, ntiles], f32)
            nc.sync.dma_start(out=srcidx_sb[:], in_=srcidx_d[:])
            nc.sync.dma_start(out=dstloc_sb[:], in_=dstloc_d[:])

            agg = bigp.tile([128, CHUNKS, 128], f32)

            # ---- Phase A: full H1s table (batches of 16 node groups) ----
            BG = 16
            for gg in range(GROUPS // BG):
                xs_t = xsp.tile([F, BG * 128], f32, tag="xs")
                nc.sync.dma_start(
                    out=xs_t[:],
                    in_=xsT_d[:, gg * BG * 128:(gg + 1) * BG * 128])
                hblk = hp.tile([128, BG, F], f32, tag="h")
                for k in range(BG):
                    ps = psg.tile([128, F], f32, tag="g")
                    nc.tensor.matmul(out=ps[:],
                                     lhsT=xs_t[:, k * 128:(k + 1) * 128],
                                     rhs=W1_t[:], start=True, stop=True)
                    nc.vector.tensor_copy(out=hblk[:, k, :], in_=ps[:])
                nc.sync.dma_start(
                    out=h1s_t[gg * BG * 128:(gg + 1) * BG * 128, :]
                        .rearrange("(k p) f -> p k f", p=128),
                    in_=hblk[:])

            # ---- Phase A2: layer-1 self term (own rows) ----
            for c in range(CHUNKS):
                xs_t = xsp.tile([F, 128], f32, tag="xs")
                nc.sync.dma_start(out=xs_t[:], in_=xso_d[:, c * 128:(c + 1) * 128])
                ps = psg.tile([128, F], f32, tag="g")
                nc.tensor.matmul(out=ps[:], lhsT=xs_t[:], rhs=W1_t[:],
                                 start=True, stop=True)
                nc.vector.tensor_scalar_mul(out=agg[:, c, :], in0=ps[:],
                                            scalar1=disT_t[:, c:c + 1])

            # ---- shared gather/scatter pass ----
            def layer_pass(table, out_stage):
                qn = 0
                for c in range(CHUNKS):
                    nb = int(spc[c]) // 128
                    gb = gbufp.tile([128, nb, 128], f32, tag="gb")
                    base_o = int(cell_off[c, 0])
                    for b in range(NBLK):
                        cnt = int(C[c, b])
                        o = int(cell_off[c, b])
                        lo = b * BLK
                        hi = min(NPAD, lo + BLK)
                        for sub in range(0, cnt, MAX_IDX_PER_CALL):
                            n = min(MAX_IDX_PER_CALL, cnt - sub)
                            ol = o - base_o + sub
                            nc.gpsimd.dma_gather(
                                gb[:, ol // 128:(ol + n) // 128, :],
                                table[lo:hi, :],
                                srcidx_sb[:, (o + sub) // 16:(o + sub + n) // 16],
                                n, n, F, queue_num=qn)
                            qn = (qn + 1) % 4
                    ps = psb.tile([128, F], f32, tag="acc")
                    base_t = base_o // 128
                    for t in range(nb):
                        oh = ohp.tile([128, 128], f32, tag="oh")
                        nc.vector.tensor_tensor(
                            out=oh[:],
                            in0=dstloc_sb[:, base_t + t:base_t + t + 1]
                                .to_broadcast([128, 128]),
                            in1=iota_r[:],
                            op=OP.is_equal)
                        nc.tensor.matmul(out=ps[:], lhsT=oh[:],
                                         rhs=gb[:, t, :],
                                         start=(t == 0), stop=(t == nb - 1))
                    out_stage(c, ps)

            # ---- Phase B: layer-1 scatter (accumulate onto self term) ----
            def b_stage(c, ps):
                t = wp.tile([128, 128], f32, tag="bs")
                nc.vector.tensor_scalar_mul(out=t[:], in0=ps[:],
                                            scalar1=disT_t[:, c:c + 1])
                nc.vector.tensor_tensor(out=agg[:, c, :], in0=t[:],
                                        in1=agg[:, c, :], op=OP.add)
            layer_pass(h1s_t.ap(), b_stage)

            # ---- Phase C: BN stats + AllReduce ----
            sum_ps = pss.tile([128, 1], f32, tag="s0")
            for c in range(CHUNKS):
                nc.tensor.matmul(out=sum_ps[:], lhsT=agg[:, c, :], rhs=ones_c[:],
                                 start=(c == 0), stop=(c == CHUNKS - 1))
            sq_ps = pss.tile([128, 1], f32, tag="s1")
            for c in range(CHUNKS):
                sq_t = wp.tile([128, 128], f32, tag="sq")
                nc.vector.tensor_tensor(out=sq_t[:], in0=agg[:, c, :],
                                        in1=agg[:, c, :], op=OP.mult)
                nc.tensor.matmul(out=sq_ps[:], lhsT=sq_t[:], rhs=ones_c[:],
                                 start=(c == 0), stop=(c == CHUNKS - 1))
            stats = stp.tile([128, 2], f32)
            nc.vector.tensor_copy(out=stats[:, 0:1], in_=sum_ps[:])
            nc.vector.tensor_copy(out=stats[:, 1:2], in_=sq_ps[:])
            nc.sync.dma_start(out=bn_in[:], in_=stats[:])
            nc.gpsimd.collective_compute(
                "AllReduce", OP.add, ins=[bn_in.ap()], outs=[bn_out.ap()],
                replica_groups=[list(range(NCORES))])
            gstats = stp.tile([128, 2], f32)
            nc.sync.dma_start(out=gstats[:], in_=bn_out[:])

            mean_t = stp.tile([128, 1], f32)
            ex2_t = stp.tile([128, 1], f32)
            var_t = stp.tile([128, 1], f32)
            sd_t = stp.tile([128, 1], f32)
            rstd_t = stp.tile([128, 1], f32)
            scale_c = stp.tile([128, 1], f32)
            shift_c = stp.tile([128, 1], f32)
            nc.vector.tensor_scalar_mul(out=mean_t[:], in0=gstats[:, 0:1],
                                        scalar1=1.0 / N)
            nc.vector.tensor_scalar_mul(out=ex2_t[:], in0=gstats[:, 1:2],
                                        scalar1=1.0 / N)
            nc.vector.tensor_tensor(out=var_t[:], in0=mean_t[:], in1=mean_t[:],
                                    op=OP.mult)
            nc.vector.tensor_tensor(out=var_t[:], in0=ex2_t[:], in1=var_t[:],
                                    op=OP.subtract)
            eps_t = stp.tile([128, 1], f32)
            nc.vector.memset(eps_t[:], BN_EPS)
            nc.scalar.activation(sd_t[:], var_t[:], AF.Sqrt, bias=eps_t[:])
            nc.vector.reciprocal(out=rstd_t[:], in_=sd_t[:])
            nc.vector.tensor_tensor(out=scale_c[:], in0=rstd_t[:], in1=gamma_t[:],
                                    op=OP.mult)
            nc.vector.tensor_tensor(out=shift_c[:], in0=mean_t[:], in1=scale_c[:],
                                    op=OP.mult)
            nc.vector.tensor_tensor(out=shift_c[:], in0=beta_t[:], in1=shift_c[:],
                                    op=OP.subtract)

            # ---- Phase D: h2 own rows, H2s table rows, layer-2 self term ----
            for c in range(CHUNKS):
                trps = psg.tile([128, 128], f32, tag="g")
                nc.tensor.transpose(out=trps[:], in_=agg[:, c, :],
                                    identity=ident[:])
                h2inT = wp.tile([128, 128], f32, tag="h2")
                nc.scalar.activation(h2inT[:], trps[:], AF.Relu,
                                     bias=shift_c[:], scale=scale_c[:])
                ps2 = psg.tile([128, 128], f32, tag="g")
                nc.tensor.matmul(out=ps2[:], lhsT=h2inT[:], rhs=W2_t[:],
                                 start=True, stop=True)
                h2s_t = hp.tile([128, F], f32, tag="h")
                nc.vector.tensor_scalar_mul(out=h2s_t[:], in0=ps2[:],
                                            scalar1=disT_t[:, c:c + 1])
                nc.sync.dma_start(out=ag_in[c * 128:(c + 1) * 128, :],
                                  in_=h2s_t[:])
                nc.vector.tensor_scalar_mul(out=agg[:, c, :], in0=h2s_t[:],
                                            scalar1=disT_t[:, c:c + 1])

            nc.gpsimd.collective_compute(
                "AllGather", OP.bypass, ins=[ag_in.ap()], outs=[ag_out.ap()],
                replica_groups=[list(range(NCORES))])

            # ---- Phase E: layer-2 scatter + bias + relu + output ----
            def e_stage(c, ps):
                t = wp.tile([128, 128], f32, tag="eo")
                nc.vector.tensor_scalar_mul(out=t[:], in0=ps[:],
                                            scalar1=disT_t[:, c:c + 1])
                nc.vector.tensor_tensor(out=t[:], in0=t[:], in1=agg[:, c, :],
                                        op=OP.add)
                nc.vector.tensor_tensor(out=t[:], in0=t[:], in1=b2m_t[:],
                                        op=OP.add)
                nc.scalar.activation(t[:], t[:], AF.Relu)
                nc.sync.dma_start(out=out_d[c * 128:(c + 1) * 128, :], in_=t[:])
            layer_pass(ag_out.ap(), e_stage)

    nc.compile()
    return nc


def kernel(**inputs):
    global LAST_EXEC_NS
    import os
    x = inputs["x"]
    W1 = np.asarray(inputs["W1"], dtype=np.float32)
    W2 = np.asarray(inputs["W2"], dtype=np.float32)
    gamma = np.asarray(inputs["gamma"], dtype=np.float32)
    beta = np.asarray(inputs["beta"], dtype=np.float32)
    b2 = np.asarray(inputs["b2"], dtype=np.float32)
    edge_index = inputs["edge_index"]

    key = (hash(np.asarray(edge_index)[:, ::997].tobytes()),)
    if key not in _cache:
        consts, xsT, per_core = _prep(x, edge_index)
        nc = _build(consts)
        _cache[key] = (consts, nc)
    else:
        consts, nc = _cache[key]
        _, xsT, per_core = _prep(x, edge_index)

    shared = {
        "xsT": xsT,
        "W1": W1, "W2": W2,
        "gamma_c": gamma.reshape(F, 1).copy(),
        "beta_c": beta.reshape(F, 1).copy(),
        "b2_mat": np.ascontiguousarray(np.broadcast_to(b2.reshape(1, F),
                                                       (128, F))),
    }
    in_maps = []
    for i in range(NCORES):
        m = dict(shared)
        m.update(per_core[i])
        in_maps.append(m)

    trace = bool(os.environ.get("BASS_GCN_TRACE"))
    res = run_bass_kernel_spmd(nc, in_maps, list(range(NCORES)), trace=trace)
    LAST_EXEC_NS = res.exec_time_ns
    global LAST_RESULT
    LAST_RESULT = res

    out = np.concatenate([res.results[i]["out"] for i in range(NCORES)], axis=0)
    return np.ascontiguousarray(out[:N]).astype(np.float32)



# revision 14
# speedup vs baseline: 1.3883x; 1.3883x over previous
"""2-layer GCN (GCNConv -> BatchNorm(train) -> ReLU -> GCNConv -> ReLU) on 8 TRN2
NeuronCores, SPMD (one NEFF on all cores).

v3 design (from NTFF profiles of v1 fp32 / v2 bf16):
  - bf16 tables / gathers / matmuls (fp32 PSUM + BN stats).
  - Gather calls merged per (super-chunk of 4 dst chunks, src block): 100
    calls/layer instead of 400.  v2 showed the Pool engine's per-call SWDGE
    cost (~1-5us) paced the whole pass while the DMA engines idled at 29%.
    Pads gather row 0 (static counts); at 256B/row the ~20% pad rows cost
    ~3us of engine time - noise.  Scatter keeps 4 open PSUM accumulators
    (one per chunk of the super) since slots are block-major.
  - ONE collective: BN stats ride the AllGather (v1 measured 511us for the
    1KB mesh AllReduce).  The payload is the TRANSPOSED pre-BN agg1 (the
    transposes are needed anyway); after the AllGather each core sums the 8
    stat header rows and builds the full private node-major h2s table with
    just activation+matmul per chunk (no transposes: lhsT IS the shipped
    feature-major layout).
  - dis[src] for layer 2 is folded into the one-hot values; dis[src] for
    layer 1 is folded into the xs table host-side.
  - Edges sorted by (cell, src) so each gather call walks ascending
    addresses (DRAM row-buffer locality).

Sharding: nodes padded 100000 -> 102400 = 8*12800, core i owns rows
[i*12800,(i+1)*12800); edges partitioned by dst owner; weights replicated.
"""
import numpy as np
import ml_dtypes

import concourse.bass as bass
import concourse.mybir as mybir
import concourse.tile as tile
from concourse import bacc
from concourse.bass_utils import run_bass_kernel_spmd
from concourse.masks import make_identity

N = 100000
F = 128
NCORES = 8
NPAD = 102400
OWN = NPAD // NCORES          # 12800
CHUNKS = OWN // 128           # 100
GCHUNKS = NPAD // 128         # 800
NBLK = 4
BLK = NPAD // NBLK            # 25600 (< 32768, int16-addressable)
SEG = OWN + 128               # 12928 AllGather segment rows (stats header)
BN_EPS = 1e-5
SC = 2                        # dst chunks per super-chunk (gather-call merge)
BF16 = ml_dtypes.bfloat16

LAST_EXEC_NS = None
LAST_RESULT = None
_cache = {}


def _prep(x, edge_index):
    src = np.asarray(edge_index[0]).astype(np.int64)
    dst = np.asarray(edge_index[1]).astype(np.int64)

    deg = np.bincount(dst, minlength=N).astype(np.float32) + 1.0
    dis = np.zeros(NPAD, dtype=np.float32)
    dis[:N] = 1.0 / np.sqrt(deg)

    xs = np.zeros((NPAD, F), dtype=np.float32)
    xs[:N] = np.asarray(x, dtype=np.float32) * dis[:N, None]
    xsT = np.ascontiguousarray(xs.T.astype(BF16))          # [F, NPAD] bf16

    owner = dst // OWN
    chunk = (dst % OWN) // 128
    blk = src // BLK
    cell = ((owner * CHUNKS + chunk) * NBLK + blk).astype(np.int64)
    order = np.lexsort((src, cell))      # ascending src within each cell
    src_s = src[order]
    dst_s = dst[order]

    counts = np.zeros((NCORES, CHUNKS, NBLK), np.int64)
    np.add.at(counts, (owner, chunk, blk), 1)
    C = counts.max(axis=0)
    C = ((C + 127) // 128) * 128
    C = np.maximum(C, 128)

    starts = np.zeros(NCORES * CHUNKS * NBLK + 1, dtype=np.int64)
    starts[1:] = np.cumsum(counts.reshape(-1))

    # super-chunk slot layout: for each super s: for each block b: the SC
    # cells (c, b) back to back.  Gather call = one (s, b) segment.
    nsup = CHUNKS // SC
    slot_pos = {}
    sup_meta = []
    off = 0
    for s in range(nsup):
        chs = list(range(s * SC, (s + 1) * SC))
        sup_off = off
        seg_calls = []
        for b in range(NBLK):
            call_off = off
            for c in chs:
                slot_pos[(c, b)] = off
                off += int(C[c, b])
            # split to <=1024 descriptors per call (SWDGE ring limit)
            seg_n = off - call_off
            sub = 0
            while sub < seg_n:
                n = min(1024, seg_n - sub)
                seg_calls.append((b, call_off + sub, n))
                sub += n
        chunk_of = []
        for b in range(NBLK):
            for ci, c in enumerate(chs):
                chunk_of.extend([ci] * (int(C[c, b]) // 128))
        first = {}
        last = {}
        for t, ci in enumerate(chunk_of):
            if ci not in first:
                first[ci] = t
            last[ci] = t
        sup_meta.append({"off": sup_off, "ntiles": len(chunk_of),
                         "chunk_of": chunk_of, "first": first, "last": last,
                         "calls": seg_calls, "chunks": chs})
    tot = off
    ntiles = tot // 128

    per_core = []
    for i in range(NCORES):
        srcidx = np.zeros(tot, dtype=np.int16)                # pads hit row 0
        dstloc = np.full(tot, -1.0, dtype=np.float32)         # pads no column
        dissrc = np.zeros(tot, dtype=np.float32)
        for c in range(CHUNKS):
            for b in range(NBLK):
                k = (i * CHUNKS + c) * NBLK + b
                m = int(counts[i, c, b])
                o = slot_pos[(c, b)]
                if m:
                    sl = slice(starts[k], starts[k] + m)
                    srcidx[o:o + m] = (src_s[sl] - b * BLK).astype(np.int16)
                    dstloc[o:o + m] = (dst_s[sl] % 128).astype(np.float32)
                    dissrc[o:o + m] = dis[src_s[sl]]
        iw = srcidx.reshape(tot // 16, 16).T                  # [16, tot/16]
        srcidx_w = np.ascontiguousarray(np.tile(iw, (8, 1)))  # [128, tot/16]
        dstloc_t = np.ascontiguousarray(
            dstloc.reshape(ntiles, 128).T.astype(BF16))
        dissrc_t = np.ascontiguousarray(
            dissrc.reshape(ntiles, 128).T.astype(BF16))
        disT = np.ascontiguousarray(
            dis[i * OWN:(i + 1) * OWN].reshape(CHUNKS, 128).T)
        xs_ownT = np.ascontiguousarray(xsT[:, i * OWN:(i + 1) * OWN])
        per_core.append({"srcidx": srcidx_w, "dstloc": dstloc_t,
                         "dissrc": dissrc_t, "disT": disT, "xs_ownT": xs_ownT})

    consts = {"tot": tot, "ntiles": ntiles, "sup_meta": sup_meta}
    return consts, xsT, per_core


def _build(consts):
    tot = consts["tot"]
    ntiles = consts["ntiles"]
    sup_meta = consts["sup_meta"]

    f32 = mybir.dt.float32
    bf16 = mybir.dt.bfloat16
    AF = mybir.ActivationFunctionType
    OP = mybir.AluOpType
    nc = bacc.Bacc("TRN2", target_bir_lowering=False, debug=False,
                   num_devices=NCORES, num_swdge_queues=4,
                   dynamic_dma_scratch_size=32768)

    xsT_d = nc.dram_tensor("xsT", [F, NPAD], bf16, kind="ExternalInput").ap()
    xso_d = nc.dram_tensor("xs_ownT", [F, OWN], bf16, kind="ExternalInput").ap()
    W1_d = nc.dram_tensor("W1b", [F, F], bf16, kind="ExternalInput").ap()
    W2_d = nc.dram_tensor("W2b", [F, F], bf16, kind="ExternalInput").ap()
    gamma_d = nc.dram_tensor("gamma_c", [F, 1], f32, kind="ExternalInput").ap()
    beta_d = nc.dram_tensor("beta_c", [F, 1], f32, kind="ExternalInput").ap()
    b2m_d = nc.dram_tensor("b2_mat", [128, F], f32, kind="ExternalInput").ap()
    disT_d = nc.dram_tensor("disT", [128, CHUNKS], f32, kind="ExternalInput").ap()
    srcidx_d = nc.dram_tensor("srcidx", [128, tot // 16], mybir.dt.int16,
                              kind="ExternalInput").ap()
    dstloc_d = nc.dram_tensor("dstloc", [128, ntiles], bf16,
                              kind="ExternalInput").ap()
    dissrc_d = nc.dram_tensor("dissrc", [128, ntiles], bf16,
                              kind="ExternalInput").ap()
    out_d = nc.dram_tensor("out", [OWN, F], f32, kind="ExternalOutput").ap()
    dbg_d = nc.dram_tensor("dbg", [128, 8], f32, kind="ExternalOutput").ap()
    dbga_d = nc.dram_tensor("dbga", [128, 128], f32, kind="ExternalOutput").ap()

    h1s = nc.dram_tensor("h1s_tab", [NPAD, F], bf16)
    h2s = nc.dram_tensor("h2s_tab", [NPAD, F], bf16)
    ag_in = nc.dram_tensor("ag_in", [SEG, F], bf16)
    ag_out = nc.dram_tensor("ag_out", [NCORES * SEG, F], bf16,
                            addr_space="Shared")

    with tile.TileContext(nc) as tc:
        with tc.tile_pool(name="const", bufs=1) as constp, \
             tc.tile_pool(name="big", bufs=1) as bigp, \
             tc.tile_pool(name="xs", bufs=2) as xsp, \
             tc.tile_pool(name="h", bufs=2) as hp, \
             tc.tile_pool(name="psA", bufs=3, space="PSUM") as psA, \
             tc.tile_pool(name="psS", bufs=SC, space="PSUM") as psS, \
             tc.tile_pool(name="pss", bufs=1, space="PSUM") as pss, \
             tc.tile_pool(name="gbuf", bufs=2) as gbufp, \
             tc.tile_pool(name="oh", bufs=6) as ohp, \
             tc.tile_pool(name="wk", bufs=4) as wp, \
             tc.tile_pool(name="st", bufs=1) as stp:

            # ---- constants ----
            W1_t = constp.tile([F, F], bf16)
            W2_t = constp.tile([F, F], bf16)
            ident_f = constp.tile([128, 128], f32)
            iota4 = constp.tile([128, 4, 128], bf16)
            ones_f = constp.tile([128, 1], f32)
            ones_b = constp.tile([128, 1], bf16)
            gamma_t = constp.tile([F, 1], f32)
            beta_t = constp.tile([F, 1], f32)
            b2m_t = constp.tile([128, F], f32)
            disT_t = constp.tile([128, CHUNKS], f32)
            disT2_t = constp.tile([128, CHUNKS], f32)
            nc.sync.dma_start(out=W1_t[:], in_=W1_d[:])
            nc.sync.dma_start(out=W2_t[:], in_=W2_d[:])
            nc.sync.dma_start(out=gamma_t[:], in_=gamma_d[:])
            nc.sync.dma_start(out=beta_t[:], in_=beta_d[:])
            nc.sync.dma_start(out=b2m_t[:], in_=b2m_d[:])
            nc.sync.dma_start(out=disT_t[:], in_=disT_d[:])
            make_identity(nc, ident_f[:])
            iota_i = constp.tile([128, 128], mybir.dt.int32)
            nc.gpsimd.iota(iota_i[:], pattern=[[1, 128]], base=0,
                           channel_multiplier=0)
            for k in range(4):
                nc.vector.tensor_copy(out=iota4[:, k, :], in_=iota_i[:])
            nc.vector.memset(ones_f[:], 1.0)
            nc.vector.memset(ones_b[:], 1.0)
            nc.vector.tensor_tensor(out=disT2_t[:], in0=disT_t[:],
                                    in1=disT_t[:], op=OP.mult)

            srcidx_sb = bigp.tile([128, tot // 16], mybir.dt.int16)
            dstloc_sb = bigp.tile([128, ntiles], bf16)
            dissrc_sb = bigp.tile([128, ntiles], bf16)
            nc.sync.dma_start(out=srcidx_sb[:], in_=srcidx_d[:])
            nc.sync.dma_start(out=dstloc_sb[:], in_=dstloc_d[:])
            nc.sync.dma_start(out=dissrc_sb[:], in_=dissrc_d[:])

            agg = bigp.tile([128, CHUNKS, 128], f32)

            # ---- Phase A: full H1s table (bf16, 16 chunks per DMA group) ----
            BG = 16
            for gg in range(GCHUNKS // BG):
                xs_t = xsp.tile([F, BG * 128], bf16, tag="xs")
                nc.sync.dma_start(
                    out=xs_t[:],
                    in_=xsT_d[:, gg * BG * 128:(gg + 1) * BG * 128])
                hblk = hp.tile([128, BG, F], bf16, tag="h")
                for q in range(BG // 4):
                    ps = psA.tile([128, 4, 128], f32, tag="a")
                    for j in range(4):
                        nc.tensor.matmul(
                            out=ps[:, j, :],
                            lhsT=xs_t[:, (q * 4 + j) * 128:(q * 4 + j + 1) * 128],
                            rhs=W1_t[:], start=True, stop=True)
                    nc.scalar.activation(hblk[:, q * 4:(q + 1) * 4, :], ps[:],
                                         AF.Copy)
                nc.sync.dma_start(
                    out=h1s[gg * BG * 128:(gg + 1) * BG * 128, :]
                        .rearrange("(k p) f -> p k f", p=128),
                    in_=hblk[:])

            # ---- Phase A2: layer-1 self term seeds agg ----
            for q in range(CHUNKS // 4):
                xs_t = xsp.tile([F, 512], bf16, tag="xs")
                nc.sync.dma_start(out=xs_t[:], in_=xso_d[:, q * 512:(q + 1) * 512])
                ps = psA.tile([128, 4, 128], f32, tag="a")
                for j in range(4):
                    nc.tensor.matmul(out=ps[:, j, :],
                                     lhsT=xs_t[:, j * 128:(j + 1) * 128],
                                     rhs=W1_t[:], start=True, stop=True)
                for j in range(4):
                    c = q * 4 + j
                    nc.vector.tensor_scalar_mul(out=agg[:, c, :],
                                                in0=ps[:, j, :],
                                                scalar1=disT_t[:, c:c + 1])

            # BN stat accumulators (separate banks)
            sum_ps = pss.tile([128, 1], f32, name="sum_ps")
            sq_ps = pss.tile([128, 1], f32, name="sq_ps")

            # ---- shared gather/scatter pass (super-chunk granularity) ----
            def layer_pass(table_ap, out_stage, l2):
                qn = 0
                for sm in sup_meta:
                    TS = sm["ntiles"]
                    gb = gbufp.tile([128, TS, 128], bf16, tag="gb")
                    base_t = sm["off"] // 128
                    for (b, coff, n) in sm["calls"]:
                        ol = coff - sm["off"]
                        nc.gpsimd.dma_gather(
                            gb[:, ol // 128:(ol + n) // 128, :],
                            table_ap[b * BLK:(b + 1) * BLK, :],
                            srcidx_sb[:, coff // 16:(coff + n) // 16],
                            n, n, F, queue_num=qn)
                        qn = (qn + 1) % 4
                    accs = [psS.tile([128, F], f32, tag="acc",
                                     name=f"acc{k}")
                            for k in range(SC)]
                    t = 0
                    while t < TS:
                        w = min(4, TS - t)
                        oh = ohp.tile([128, 4, 128], bf16, tag="oh")
                        nc.vector.tensor_tensor(
                            out=oh[:, :w, :],
                            in0=dstloc_sb[:, base_t + t:base_t + t + w]
                                .to_broadcast([128, w, 128]),
                            in1=iota4[:, :w, :], op=OP.is_equal)
                        if l2:
                            nc.vector.tensor_tensor(
                                out=oh[:, :w, :], in0=oh[:, :w, :],
                                in1=dissrc_sb[:, base_t + t:base_t + t + w]
                                    .to_broadcast([128, w, 128]),
                                op=OP.mult)
                        for j in range(w):
                            ci = sm["chunk_of"][t + j]
                            nc.tensor.matmul(out=accs[ci][:],
                                             lhsT=oh[:, j, :],
                                             rhs=gb[:, t + j, :],
                                             start=(sm["first"][ci] == t + j),
                                             stop=(sm["last"][ci] == t + j))
                        t += w
                    for ci, c in enumerate(sm["chunks"]):
                        out_stage(c, accs[ci])

            # ---- L1 scatter: agg += dis_dst * ps ----
            def b_stage(c, ps):
                tt = wp.tile([128, 128], f32, tag="bs")
                nc.vector.tensor_scalar_mul(out=tt[:], in0=ps[:],
                                            scalar1=disT_t[:, c:c + 1])
                nc.vector.tensor_tensor(out=agg[:, c, :], in0=tt[:],
                                        in1=agg[:, c, :], op=OP.add)
            layer_pass(h1s.ap(), b_stage, l2=False)

            # ---- BN stats (contiguous accumulation, baseline-proven) ----
            for c in range(CHUNKS):
                nc.tensor.matmul(out=sum_ps[:], lhsT=agg[:, c, :],
                                 rhs=ones_f[:],
                                 start=(c == 0), stop=(c == CHUNKS - 1))
            for c in range(CHUNKS):
                sq = wp.tile([128, 128], bf16, tag="sq")
                nc.scalar.square(sq[:], agg[:, c, :])
                nc.tensor.matmul(out=sq_ps[:], lhsT=sq[:], rhs=ones_b[:],
                                 start=(c == 0), stop=(c == CHUNKS - 1))
            nc.sync.dma_start(out=dbga_d[:], in_=agg[:, 0, :])

            # ---- stats header -> ag_in rows [0,128) (rows 0,1 used) ----
            stats2 = stp.tile([128, 2], f32)
            nc.vector.tensor_copy(out=stats2[:, 0:1], in_=sum_ps[:])
            nc.vector.tensor_copy(out=stats2[:, 1:2], in_=sq_ps[:])
            stpad = stp.tile([128, 128], f32)
            nc.vector.memset(stpad[:], 0.0)
            nc.vector.tensor_copy(out=stpad[:, 0:2], in_=stats2[:])
            trs = psA.tile([128, 4, 128], f32, tag="a")
            nc.tensor.transpose(out=trs[:, 0, :], in_=stpad[:],
                                identity=ident_f[:])
            stag = stp.tile([128, 128], bf16)
            nc.scalar.activation(stag[:], trs[:, 0, :], AF.Copy)
            nc.sync.dma_start(out=ag_in[0:128, :], in_=stag[:])

            # ---- transpose agg1 (needed for layer 2 anyway), ship agg1^T ----
            for q in range(CHUNKS // 4):
                trp = psA.tile([128, 4, 128], f32, tag="a")
                for j in range(4):
                    nc.tensor.transpose(out=trp[:, j, :],
                                        in_=agg[:, q * 4 + j, :],
                                        identity=ident_f[:])
                tst = wp.tile([128, 4, 128], bf16, tag="tT")
                nc.scalar.activation(tst[:], trp[:], AF.Copy)
                nc.sync.dma_start(
                    out=ag_in[128 + q * 512:128 + (q + 1) * 512, :]
                        .rearrange("(k p) f -> p k f", p=128),
                    in_=tst[:])

            nc.gpsimd.collective_compute(
                "AllGather", OP.bypass, ins=[ag_in.ap()], outs=[ag_out.ap()],
                replica_groups=[list(range(NCORES))])

            # ---- global BN stats from the 8 headers ----
            # (two plain DMAs: a partition-dim rearrange on the SBUF side of
            # a DMA silently misplaces rows - learned the hard way)
            gst = stp.tile([16, 128], bf16)
            agv = ag_out.ap().rearrange("(i s) f -> i s f", i=NCORES)
            nc.sync.dma_start(out=gst[0:8, :], in_=agv[:, 0, :])
            nc.sync.dma_start(out=gst[8:16, :], in_=agv[:, 1, :])
            gpad = stp.tile([128, 128], f32)
            nc.vector.memset(gpad[:], 0.0)
            nc.vector.tensor_copy(out=gpad[0:16, :], in_=gst[:])
            gtr = psA.tile([128, 4, 128], f32, tag="a")
            nc.tensor.transpose(out=gtr[:, 0, :], in_=gpad[:],
                                identity=ident_f[:])
            # cols 0..7 = per-core sums, 8..15 = per-core sumsqs
            gred = stp.tile([128, 16], f32)
            nc.vector.tensor_copy(out=gred[:], in_=gtr[:, 0, 0:16])
            nc.vector.tensor_tensor(out=gred[:, 0:4], in0=gred[:, 0:4],
                                    in1=gred[:, 4:8], op=OP.add)
            nc.vector.tensor_tensor(out=gred[:, 8:12], in0=gred[:, 8:12],
                                    in1=gred[:, 12:16], op=OP.add)
            nc.vector.tensor_tensor(out=gred[:, 0:2], in0=gred[:, 0:2],
                                    in1=gred[:, 2:4], op=OP.add)
            nc.vector.tensor_tensor(out=gred[:, 8:10], in0=gred[:, 8:10],
                                    in1=gred[:, 10:12], op=OP.add)
            nc.vector.tensor_tensor(out=gred[:, 0:1], in0=gred[:, 0:1],
                                    in1=gred[:, 1:2], op=OP.add)
            nc.vector.tensor_tensor(out=gred[:, 8:9], in0=gred[:, 8:9],
                                    in1=gred[:, 9:10], op=OP.add)

            mean_t = stp.tile([128, 1], f32)
            ex2_t = stp.tile([128, 1], f32)
            var_t = stp.tile([128, 1], f32)
            sd_t = stp.tile([128, 1], f32)
            rstd_t = stp.tile([128, 1], f32)
            scale_c = stp.tile([128, 1], f32)
            shift_c = stp.tile([128, 1], f32)
            eps_t = stp.tile([128, 1], f32)
            nc.vector.tensor_scalar_mul(out=mean_t[:], in0=gred[:, 0:1],
                                        scalar1=1.0 / N)
            nc.vector.tensor_scalar_mul(out=ex2_t[:], in0=gred[:, 8:9],
                                        scalar1=1.0 / N)
            nc.vector.tensor_tensor(out=var_t[:], in0=mean_t[:], in1=mean_t[:],
                                    op=OP.mult)
            nc.vector.tensor_tensor(out=var_t[:], in0=ex2_t[:], in1=var_t[:],
                                    op=OP.subtract)
            nc.vector.tensor_scalar_max(out=var_t[:], in0=var_t[:],
                                        scalar1=0.0)
            nc.vector.memset(eps_t[:], BN_EPS)
            nc.scalar.activation(sd_t[:], var_t[:], AF.Sqrt, bias=eps_t[:])
            nc.vector.reciprocal(out=rstd_t[:], in_=sd_t[:])
            nc.vector.tensor_tensor(out=scale_c[:], in0=rstd_t[:],
                                    in1=gamma_t[:], op=OP.mult)
            nc.vector.tensor_tensor(out=shift_c[:], in0=mean_t[:],
                                    in1=scale_c[:], op=OP.mult)
            nc.vector.tensor_tensor(out=shift_c[:], in0=beta_t[:],
                                    in1=shift_c[:], op=OP.subtract)
            dbg_t = stp.tile([128, 8], f32)
            nc.vector.tensor_copy(out=dbg_t[:, 0:2], in_=stats2[:])
            nc.vector.tensor_copy(out=dbg_t[:, 2:3], in_=gred[:, 0:1])
            nc.vector.tensor_copy(out=dbg_t[:, 3:4], in_=gred[:, 8:9])
            nc.vector.tensor_copy(out=dbg_t[:, 4:5], in_=var_t[:])
            nc.vector.tensor_copy(out=dbg_t[:, 5:6], in_=scale_c[:])
            nc.vector.tensor_copy(out=dbg_t[:, 6:7], in_=shift_c[:])
            nc.vector.tensor_copy(out=dbg_t[:, 7:8], in_=mean_t[:])
            nc.sync.dma_start(out=dbg_d[:], in_=dbg_t[:])

            # ---- layer-2 self-term seeds (own agg1^T read back from the
            #      private ag_in copy; per-core static address) ----
            for q in range(CHUNKS // 4):
                stb = xsp.tile([128, 4, 128], bf16, tag="cb")
                nc.sync.dma_start(
                    out=stb[:],
                    in_=ag_in.ap()[128 + q * 512:128 + (q + 1) * 512, :]
                        .rearrange("(k p) f -> p k f", p=128))
                h2in = wp.tile([128, 4, 128], bf16, tag="h2")
                nc.scalar.activation(h2in[:], stb[:],
                                     AF.Relu, bias=shift_c[:], scale=scale_c[:])
                ps2 = psA.tile([128, 4, 128], f32, tag="a")
                for j in range(4):
                    nc.tensor.matmul(out=ps2[:, j, :], lhsT=h2in[:, j, :],
                                     rhs=W2_t[:], start=True, stop=True)
                for j in range(4):
                    c = q * 4 + j
                    nc.vector.tensor_scalar_mul(out=agg[:, c, :],
                                                in0=ps2[:, j, :],
                                                scalar1=disT2_t[:, c:c + 1])
                    nc.vector.tensor_tensor(out=agg[:, c, :], in0=agg[:, c, :],
                                            in1=b2m_t[:], op=OP.add)

            # ---- build full private h2s table from ag_out (feature-major
            #      payload: no transposes needed) ----
            for i in range(NCORES):
                for g0 in range(0, CHUNKS, 4):
                    ctb = xsp.tile([128, 4, 128], bf16, tag="cb")
                    rows0 = i * SEG + 128 + g0 * 128
                    nc.sync.dma_start(
                        out=ctb[:],
                        in_=ag_out.ap()[rows0:rows0 + 512, :]
                            .rearrange("(k p) f -> p k f", p=128))
                    h2in = wp.tile([128, 4, 128], bf16, tag="h2")
                    nc.scalar.activation(h2in[:], ctb[:], AF.Relu,
                                         bias=shift_c[:], scale=scale_c[:])
                    ps2 = psA.tile([128, 4, 128], f32, tag="a")
                    for j in range(4):
                        nc.tensor.matmul(out=ps2[:, j, :], lhsT=h2in[:, j, :],
                                         rhs=W2_t[:], start=True, stop=True)
                    hb2 = hp.tile([128, 4, F], bf16, tag="h")
                    nc.vector.tensor_copy(out=hb2[:], in_=ps2[:])
                    orow = i * OWN + g0 * 128
                    nc.sync.dma_start(
                        out=h2s[orow:orow + 512, :]
                            .rearrange("(k p) f -> p k f", p=128),
                        in_=hb2[:])

            # ---- layer-2 scatter + relu + output ----
            def e_stage(c, ps):
                tt = wp.tile([128, 128], f32, tag="eo")
                nc.vector.tensor_scalar_mul(out=tt[:], in0=ps[:],
                                            scalar1=disT_t[:, c:c + 1])
                nc.vector.tensor_tensor(out=tt[:], in0=tt[:], in1=agg[:, c, :],
                                        op=OP.add)
                ot = wp.tile([128, 128], f32, tag="ot")
                nc.scalar.activation(ot[:], tt[:], AF.Relu)
                nc.sync.dma_start(out=out_d[c * 128:(c + 1) * 128, :], in_=ot[:])
            layer_pass(h2s.ap(), e_stage, l2=True)

    nc.compile()
    return nc


def kernel(**inputs):
    global LAST_EXEC_NS, LAST_RESULT
    import os
    x = inputs["x"]
    W1 = np.asarray(inputs["W1"], dtype=np.float32)
    W2 = np.asarray(inputs["W2"], dtype=np.float32)
    gamma = np.asarray(inputs["gamma"], dtype=np.float32)
    beta = np.asarray(inputs["beta"], dtype=np.float32)
    b2 = np.asarray(inputs["b2"], dtype=np.float32)
    edge_index = inputs["edge_index"]

    key = (hash(np.asarray(edge_index)[:, ::997].tobytes()),)
    if key not in _cache:
        consts, xsT, per_core = _prep(x, edge_index)
        nc = _build(consts)
        _cache[key] = (consts, nc)
    else:
        consts, nc = _cache[key]
        _, xsT, per_core = _prep(x, edge_index)

    shared = {
        "xsT": xsT,
        "W1b": W1.astype(BF16), "W2b": W2.astype(BF16),
        "gamma_c": gamma.reshape(F, 1).copy(),
        "beta_c": beta.reshape(F, 1).copy(),
        "b2_mat": np.ascontiguousarray(np.broadcast_to(b2.reshape(1, F),
                                                       (128, F))),
    }
    in_maps = []
    for i in range(NCORES):
        m = dict(shared)
        m.update(per_core[i])
        in_maps.append(m)

    trace = bool(os.environ.get("BASS_GCN_TRACE"))
    res = run_bass_kernel_spmd(nc, in_maps, list(range(NCORES)), trace=trace)
    LAST_EXEC_NS = res.exec_time_ns
    LAST_RESULT = res

    out = np.concatenate([res.results[i]["out"] for i in range(NCORES)], axis=0)
    return np.ascontiguousarray(out[:N]).astype(np.float32)
